# revision 37
# baseline (speedup 1.0000x reference)
"""Batched cosine-similarity matrix (retrieval_knn) on 8 TRN2 NeuronCores.

reference:  out[b, n, m] = <x[b,n,:], y[b,m,:]> / max(||x[b,n]|| * ||y[b,m]||, 1e-8)
shapes:     x, y: [8, 2048, 512] f32  ->  out: [8, 2048, 2048] f32

Sharding: data-parallel over the batch dim -- batch b runs on core b.
Each core receives x[b] and y[b] in bf16, host-packed into one
DMA-priority-ordered tensor (bf16 rounding of inputs/outputs costs
~5.6e-3 max-rel error vs the harness's 2e-2 gate).

Design notes (~81.5us vs the f32r baseline's 132.4us):
  * bf16 matmuls: FWL fast-weight-load + the PE's LDW pull-ahead hide
    the per-matmul weight reload that cost ~190ns each at f32r; 512-free
    matmuls sustain ~216-228ns (2.4 GHz roofline is 213ns).
  * ~36 warm-up matmuls on a constant tile run during the input-DMA
    window so the PE_HAM clock gate (cold = 1.2 GHz, warm = 2.4 GHz) is
    released before the real matmuls start.  The f32r baseline ran most
    of its matmuls at 427ns (1.2 GHz) instead of ~216ns.
  * bf16 halves both input DMA (8.4 -> 4.2 MB/core) and output DMA
    (16.8 -> 8.4 MB/core); the host upcasts the bf16 result to f32.
  * inputs arrive as 3 contiguous DMAs in need-order (x_c0+y_c0 first --
    SDMA engines round-robin across live queues, so the critical first
    1MB must be a single DMA to get full HBM bandwidth); outputs leave
    as one [128, 2048] row-DMA per tile-row (4KB lines, 16 issues
    instead of 64 -- descriptor issue costs ~617ns of sequencer time).
  * epilogue (PSUM -> SBUF with per-row rx / per-col ry scaling) is
    split: rows on DVE (one fused scalar_tensor_tensor) except rows
    5,7,9,11,13 which go ACT scale-copy (x rx) + GpSimd tensor-mult
    (x ry), so no single engine gates the PE; the last row alternates
    engines per tile + two half-row DMAs to shorten the drain tail.
  * squares are spread over DVE/ACT/GpSimd by arrival time and engine
    speed; 1/sqrt = ACT sqrt + DVE reciprocal_approx_fast (the exact
    DVE reciprocal is 6.4 cyc/elem, ~3.3us per [128,512] chunk).
  * rx (row norms of x) via 16 N=1 matmuls per 4-tile group, PSUM-
    accumulated over k; ry (col norms of y) via ones.T @ ysq matmuls.
"""

import numpy as np

import concourse.bass as bass
import concourse.bacc as bacc
import concourse.mybir as mybir
import concourse.tile as tile
from concourse.bass_utils import run_bass_kernel_spmd

P = 128          # partitions
D = 512          # feature dim (contraction)
N = 2048         # rows of x / y
B = 8            # batch == n_cores
KC = D // P      # 4 k-chunks
NT = N // P      # 16 n-tiles (output partition tiles)
MC = N // 512    # 4 m-chunks (output free chunks, PSUM-bank width)
NG = 4           # rx groups (4 t-tiles each, one per x column-chunk)
WARM = 24        # HAM warm-up matmuls ([128,256], ~214ns cold / ~109ns warm)

F32 = mybir.dt.float32
BF16 = mybir.dt.bfloat16
BF16_NP = mybir.dt.np(mybir.dt.bfloat16)

_CACHED = {}


def _build_nc() -> bass.Bass:
    """Build the single-core Bass program (same program runs SPMD on 8 cores)."""
    nc = bacc.Bacc(trn_type="TRN2", target_bir_lowering=False, debug=False)

    # Packed host layout: x and y interleaved in DMA-priority order --
    # [x_c0 | y_c0 | y_c1 | y_c2 | y_c3 | x_c1 | x_c2 | x_c3], each block
    # [128, 2048] with (k, j) minor layout: block[p, k*512 + j] =
    # a[c*512 + j, k*128 + p].  The first 1MB (x_c0 + y_c0 = everything
    # tile (0,0) needs) is then ONE contiguous full-bandwidth DMA.
    xyP = nc.dram_tensor("xyP", [P, 2 * KC * N], BF16,
                         kind="ExternalInput").ap()
    out = nc.dram_tensor("out", [N, N], BF16, kind="ExternalOutput").ap()

    def xbase(c):
        return 0 if c == 0 else (4 + c) * 2048

    def ybase(c):
        return (1 + c) * 2048

    def xoff(t, k):
        # lhsT column block for output tile-row t, contraction chunk k
        return xbase(t // 4) + k * 512 + (t % 4) * P

    def yoff(k, c):
        # rhs column block for output col-chunk c, contraction chunk k
        return ybase(c) + k * 512

    mul = mybir.AluOpType.mult
    COPY_FN = mybir.ActivationFunctionType.Copy

    with tile.TileContext(nc) as tc:
        with (
            tc.tile_pool(name="xin", bufs=1) as xin_pool,
            tc.tile_pool(name="yin", bufs=1) as yin_pool,
            tc.tile_pool(name="sq", bufs=1) as sq_pool,
            tc.tile_pool(name="consts", bufs=1) as const_pool,
            tc.tile_pool(name="norms", bufs=1) as norm_pool,
            tc.tile_pool(name="ostage", bufs=6) as out_pool,
            tc.tile_pool(name="tmp1", bufs=9) as tmp_pool,
            tc.tile_pool(name="mm_ps", bufs=6, space="PSUM") as mm_ps_pool,
            tc.tile_pool(name="norm_ps", bufs=2, space="PSUM") as norm_ps_pool,
        ):
            # ---- constants --------------------------------------------
            # junk feeds the warm-up matmuls; memset first so the PE
            # dummies start as soon as possible after the preamble.
            junk = const_pool.tile([P, 256], BF16, name="junk")
            nc.vector.memset(junk, 1.0)
            ones_b = const_pool.tile([P, 512], BF16, name="ones_b")
            nc.vector.memset(ones_b, 1.0)

            # ---- HAM warm-up: keep the PE busy while inputs stream in
            for i in range(WARM):
                wp = mm_ps_pool.tile([P, 256], F32, name="warm", tag="ps")
                nc.tensor.matmul(wp, lhsT=junk[:, 0:P], rhs=junk,
                                 start=True, stop=True)

            # ---- input DMAs (sync/HWDGE), arrival-priority order ------
            # DMA 1: x_c0 + y_c0 together (1MB, everything tile (0,0)
            # needs) at full bandwidth; DMA 2: y_c1..c3; DMA 3: x_c1..c3
            # (x chunk c is only needed from tile-row 4c on).
            xy = xin_pool.tile([P, 2 * KC * N], BF16, name="xy", tag="xy")

            nc.sync.dma_start(out=xy[:, 0:4096], in_=xyP[:, 0:4096])
            nc.sync.dma_start(out=xy[:, 4096:10240], in_=xyP[:, 4096:10240])
            nc.sync.dma_start(out=xy[:, 10240:16384], in_=xyP[:, 10240:16384])

            # ---- squares: split across GpSimd / ACT / DVE so the ry/rx
            # chains are ready when the in-order PE queue reaches them.
            xysq = sq_pool.tile([P, 2 * KC * N], BF16, name="xysq", tag="xysq")

            def squares(eng, base):
                blk = slice(base, base + 2048)
                if eng is nc.scalar:
                    eng.square(xysq[:, blk], xy[:, blk])
                else:
                    eng.tensor_tensor(xysq[:, blk], xy[:, blk],
                                      xy[:, blk], mul)

            # ---- norm tensors -----------------------------------------
            sny = norm_pool.tile([P, N], F32, name="sny")
            ry = norm_pool.tile([P, N], F32, name="ry")
            ry_b = norm_pool.tile([P, N], BF16, name="ry_b")
            rx_sqrt = norm_pool.tile([P, NT], F32, name="rx_sqrt")
            rx = norm_pool.tile([P, NT], F32, name="rx")

            def ry_mms(c):
                n_ps = norm_ps_pool.tile([P, 512], F32, name="n_ps", tag="n_ps")
                for k in range(KC):
                    o = yoff(k, c)
                    nc.tensor.matmul(n_ps, lhsT=ones_b[:, 0:P],
                                     rhs=xysq[:, o:o + 512],
                                     start=(k == 0), stop=(k == KC - 1))
                return n_ps

            def ry_finish(c, n_ps):
                cs = slice(c * 512, (c + 1) * 512)
                nc.scalar.sqrt(sny[:, cs], n_ps)
                nc.vector.reciprocal_approx_fast(ry[:, cs], sny[:, cs])

            def rx_group(g):
                # rx[:, 4g:4g+4] = 1/sqrt(col-sums of xsq t-tiles 4g..4g+3)
                gs = slice(4 * g, 4 * g + 4)
                r_ps = norm_ps_pool.tile([P, NG], F32, name="r_ps", tag="n_ps")
                for tt in range(4):
                    t = 4 * g + tt
                    for k in range(KC):
                        o = xoff(t, k)
                        nc.tensor.matmul(
                            r_ps[:, tt:tt + 1],
                            lhsT=xysq[:, o:o + P],
                            rhs=ones_b[:, 0:1],
                            start=(k == 0), stop=(k == KC - 1),
                            skip_group_check=True,
                        )
                nc.scalar.sqrt(rx_sqrt[:, gs], r_ps)
                nc.vector.reciprocal_approx_fast(rx[:, gs], rx_sqrt[:, gs])

            def tile_mms(t, c):
                ps = mm_ps_pool.tile([P, 512], F32, name="ps", tag="ps")
                for k in range(KC):
                    xo = xoff(t, k)
                    yo = yoff(k, c)
                    nc.tensor.matmul(ps, lhsT=xy[:, xo:xo + P],
                                     rhs=xy[:, yo:yo + 512],
                                     start=(k == 0), stop=(k == KC - 1))
                return ps

            def epi_dve(t, c, ps, ot):
                cs = slice(c * 512, (c + 1) * 512)
                nc.vector.scalar_tensor_tensor(
                    ot[:, cs], in0=ps, scalar=rx[:, t:t + 1], in1=ry[:, cs],
                    op0=mul, op1=mul,
                )

            def epi_stage1(t, c, ps):
                # ACT: PSUM -> SBUF, x rx.  Only needs rx (not ry) -- used
                # for rows 0/1 to free PSUM banks before ry is ready.
                tmp = tmp_pool.tile([P, 512], BF16, name="tmp", tag="tmp")
                nc.scalar.activation(tmp, ps, COPY_FN, scale=rx[:, t:t + 1])
                return tmp

            def epi_stage2(c, tmp, ot):
                cs = slice(c * 512, (c + 1) * 512)
                nc.gpsimd.tensor_tensor(ot[:, cs], tmp, ry_b[:, cs], mul)

            def epi_act_gp(t, c, ps, ot):
                epi_stage2(c, epi_stage1(t, c, ps), ot)

            ACT_GP_ROWS = {5, 7, 9, 11, 13}

            def out_row(t, ot):
                ts_ = slice(t * P, (t + 1) * P)
                nc.scalar.dma_start(out=out[ts_, :], in_=ot)

            def full_row(t):
                pss = [tile_mms(t, c) for c in range(MC)]
                ot = out_pool.tile([P, N], BF16, name="ot", tag="ot")
                for c in range(MC):
                    if t in ACT_GP_ROWS:
                        epi_act_gp(t, c, pss[c], ot)
                    else:
                        epi_dve(t, c, pss[c], ot)
                out_row(t, ot)

            # ---- prologue: squares assigned by arrival time and engine
            # speed (DVE 424ns/chunk, ACT 612ns, GpSimd 1007ns per 512-col).
            squares(nc.vector, xbase(0))       # DVE: xsq c0 (rx_g0 feed)
            squares(nc.scalar, ybase(0))       # ACT: ysq c0
            squares(nc.scalar, ybase(1))       # ACT: ysq c1
            squares(nc.vector, ybase(2))       # DVE: ysq c2
            squares(nc.vector, ybase(3))       # DVE: ysq c3
            squares(nc.gpsimd, xbase(1))       # GpSimd: xsq c1 (rx_g1)
            squares(nc.gpsimd, xbase(2))       # GpSimd: xsq c2 (rx_g2)
            squares(nc.gpsimd, xbase(3))       # GpSimd: xsq c3 (rx_g3)

            ps00 = tile_mms(0, 0)
            ps01 = tile_mms(0, 1)
            rx_group(0)
            nps0 = ry_mms(0)
            ry_finish(0, nps0)
            ps02 = tile_mms(0, 2)
            ps03 = tile_mms(0, 3)
            for c in range(1, MC):
                ry_finish(c, ry_mms(c))

            ot0 = out_pool.tile([P, N], BF16, name="ot", tag="ot")
            for c, ps in enumerate([ps00, ps01, ps02, ps03]):
                epi_dve(0, c, ps, ot0)
            out_row(0, ot0)

            full_row(1)
            full_row(2)
            full_row(3)
            rx_group(1)
            # ry in bf16 for the GpSimd epilogue path
            for c in range(MC):
                cs = slice(c * 512, (c + 1) * 512)
                nc.gpsimd.tensor_copy(ry_b[:, cs], ry[:, cs])
            full_row(4)
            full_row(5)
            full_row(6)
            rx_group(2)
            full_row(7)
            full_row(8)
            full_row(9)
            rx_group(3)
            for t in range(10, NT - 1):
                full_row(t)

            # last row: alternate engines per tile + two half-row DMAs so
            # the drain tail after the final matmul is as short as possible
            # (the last tile's epilogue is the single-pass DVE one).
            t = NT - 1
            pss = [tile_mms(t, c) for c in range(MC)]
            ot = out_pool.tile([P, N], BF16, name="ot", tag="ot")
            epi_dve(t, 0, pss[0], ot)
            epi_act_gp(t, 1, pss[1], ot)
            nc.scalar.dma_start(out=out[t * P:(t + 1) * P, 0:1024],
                                in_=ot[:, 0:1024])
            epi_dve(t, 2, pss[2], ot)
            epi_dve(t, 3, pss[3], ot)
            nc.scalar.dma_start(out=out[t * P:(t + 1) * P, 1024:N],
                                in_=ot[:, 1024:N])

    nc.compile()
    return nc


def _get_nc(mm_dtype: str = "bfloat16") -> bass.Bass:
    if mm_dtype not in _CACHED:
        _CACHED[mm_dtype] = _build_nc()
    return _CACHED[mm_dtype]


def _pack(a: np.ndarray) -> np.ndarray:
    """[2048, 512] -> [128, 8192] with layout [p, c*2048 + k*512 + j] =
    a[c*512 + j, k*128 + p], so each 512-row chunk is one contiguous DMA."""
    v = a.reshape(MC, 512, KC, P)            # [c, j, k, p]
    return np.ascontiguousarray(
        v.transpose(3, 0, 2, 1).reshape(P, KC * N)).astype(BF16_NP)


def _shard(x: np.ndarray, y: np.ndarray):
    """Host-side sharding: batch b -> core b, bf16, chunk-packed and
    interleaved in DMA-priority order [x_c0 | y_c0..c3 | x_c1..c3]."""
    x = np.asarray(x, dtype=np.float32)
    y = np.asarray(y, dtype=np.float32)
    maps = []
    for b in range(B):
        xp = _pack(x[b])
        yp = _pack(y[b])
        xy = np.ascontiguousarray(
            np.concatenate([xp[:, :2048], yp, xp[:, 2048:]], axis=1))
        maps.append({"xyP": xy})
    return maps


def _run(x: np.ndarray, y: np.ndarray, mm_dtype: str = "bfloat16",
         trace: bool = False):
    """Returns (out [8, 2048, 2048] f32, BassKernelResults)."""
    nc = _get_nc(mm_dtype)
    in_maps = _shard(x, y)
    res = run_bass_kernel_spmd(nc, in_maps, core_ids=list(range(B)), trace=trace)
    out = np.stack([res.results[b]["out"].astype(np.float32) for b in range(B)])
    return out, res


def kernel(x: np.ndarray, y: np.ndarray) -> np.ndarray:
    out, _ = _run(x, y)
    return out
